# revision 6
# baseline (speedup 1.0000x reference)
"""CoGOL ordinal-logistic loss on 8 Trainium2 NeuronCores.

Math (per sample, target t in [1,64], logits x[0..62], cum=[0|x]):
  loss_i = sum_{j<=t-3} log_sigmoid(-x_j) + sum_{t-1<=j<=61} log_sigmoid(x_j)
           + [t>=2]*log_sigmoid(0)            (col 0 of cum; x_62 never used)
With s = clip(t-2-j, -1, 1):  the two masked sums equal
  -[ sum_{j=0}^{61} softplus(s_j * x_j) - ln2 * [2<=t<=63] ]
so with N64 = count(t==64) per core:
  loss_core = -sum softplus(s*x) - ln2 * N64
and the final result is -loss/B + a/2*sum(w^2) + b/2*sum(d[1:]^2).

Sharding: batch split 8 ways (65536 rows/core); weights flat-split 8 ways;
deltas[1:] to core 0 only (others get zeros). Each core emits one partial
scalar; host sums the 8 partials.

Perf notes (v3):
- Partition p owns rows p*512..p*512+511 of the core shard; tile k covers
  subrows k*R..k*R+R-1, so every logits DMA descriptor is one contiguous
  read per partition and target slices come from one up-front load.
- DVE rate model (measured): tensor ops stream 2 bf16/lane/cycle only
  when operands are unit-stride; a 0-step (broadcast) on the innermost
  dim halves TT to f32 rate, but a 0-step on an outer dim is free. So s
  is built in transposed [p, c, m] layout (t broadcast rides the middle
  dim) and consumed by the multiply through a strided view.
- softplus = Ln(Exp(arg) + 1) on the ACT engine (no fused softplus in
  the activation tables); bf16 in/out with f32 accumulators. Exp/Ln
  share one table (pinned below).
"""

import sys

sys.path.insert(0, "/opt/trn_rl_repo")

import numpy as np

ALPHA = 0.01
BETA = 0.05
B = 524288
KM1 = 63
KC = 62                     # columns actually used (x_62 unused)
NCORES = 8
BC = B // NCORES            # 65536 rows per core
R = 64                      # subrows per partition per tile
NT = 8                      # tiles; NT*R = 512 rows per partition
RTOT = NT * R
WPER = (3 * 512 * 512) // NCORES  # 98304 weights elements per core
LN2 = 0.6931471805599453

USE_T = False               # transposed s layout loses: strided TT is 74G

_PROG = None


def _build():
    import concourse.bacc as bacc
    import concourse.tile as tile
    from concourse import mybir

    import concourse.hw_specs as hw_specs
    if not getattr(bacc, "_act_tables_pinned", False):
        _orig_get = hw_specs.get_activation_tables

        def _pinned(arch, _orig=_orig_get):
            tabs = _orig(arch)
            keep = "natural_log_exp_and_others"
            return {k: (v if k == keep else set()) for k, v in tabs.items()}

        bacc.get_activation_tables = _pinned
        bacc._act_tables_pinned = True

    f32 = mybir.dt.float32
    bf16 = mybir.dt.bfloat16
    i32 = mybir.dt.int32
    Alu = mybir.AluOpType
    Act = mybir.ActivationFunctionType

    nc = bacc.Bacc("TRN2", target_bir_lowering=False, debug=False, num_devices=NCORES)

    logits = nc.dram_tensor("logits", [BC, KM1], f32, kind="ExternalInput")
    targets = nc.dram_tensor("targets", [BC], f32, kind="ExternalInput")
    wts = nc.dram_tensor("wts", [WPER], f32, kind="ExternalInput")
    dls = nc.dram_tensor("dls", [192], f32, kind="ExternalInput")
    out = nc.dram_tensor("out", [1, 1], f32, kind="ExternalOutput")

    lg4 = logits.ap().rearrange("(p q m) c -> q p m c", p=128, q=NT, m=R)

    with tile.TileContext(nc) as tc:
        with (
            tc.tile_pool(name="const", bufs=1) as cpool,
            tc.tile_pool(name="x", bufs=3) as xpool,
            tc.tile_pool(name="b", bufs=2) as bpool,
            tc.tile_pool(name="w", bufs=2) as wpool,
            tc.tile_pool(name="a", bufs=2) as apool,
            tc.tile_pool(name="e", bufs=2) as epool,
            tc.tile_pool(name="side", bufs=1) as spool,
            tc.tile_pool(name="fin", bufs=1) as fpool,
            tc.tile_pool(name="ps", bufs=1, space="PSUM") as ppool,
        ):
            ones = cpool.tile([128, 1], f32)
            nc.vector.memset(ones[:], 1.0)

            # all targets up-front: T[p, r] = targets[p*512 + r]
            tload = cpool.tile([128, RTOT], f32)
            nc.sync.dma_start(
                tload[:], targets.ap().rearrange("(p r) -> p r", p=128)
            )
            tb = cpool.tile([128, RTOT], bf16)
            nc.vector.tensor_copy(tb[:], tload[:])

            # iota j+2 constants
            iota_i = cpool.tile([128, KC], i32)
            nc.gpsimd.iota(iota_i[:], pattern=[[1, KC]], base=2,
                           channel_multiplier=0)
            iota_b = cpool.tile([128, KC], bf16)
            nc.vector.tensor_copy(iota_b[:], iota_i[:])
            if USE_T:
                # [p, c, m] materialized iota (innermost-0step copy, once)
                iota_T = cpool.tile([128, KC, R], bf16)
                nc.vector.tensor_copy(
                    iota_T[:], iota_b[:][:, :, None].to_broadcast([128, KC, R]))
            else:
                iota_F = cpool.tile([128, R, KC], bf16)
                nc.vector.tensor_copy(
                    iota_F[:], iota_b[:][:, None, :].to_broadcast([128, R, KC]))

            acc = cpool.tile([128, NT], f32)

            for k in range(NT):
                xt = xpool.tile([128, R, KM1], f32, tag="x")
                nc.sync.dma_start(xt[:], lg4[k])

                tsl = tb[:, k * R:(k + 1) * R]
                if USE_T:
                    # s in [p, c, m]: t bcast on middle dim runs full rate
                    wt = wpool.tile([128, KC, R], bf16, tag="w")
                    nc.vector.tensor_copy(
                        wt[:], tsl[:, None, :].to_broadcast([128, KC, R]))
                    nc.vector.tensor_tensor(
                        wt[:], wt[:], iota_T[:], Alu.subtract)
                    nc.vector.tensor_scalar(
                        wt[:], wt[:], -1.0, 1.0, Alu.max, Alu.min)
                    sview = wt[:].rearrange("p c r -> p r c")
                else:
                    wt = wpool.tile([128, R, KC], bf16, tag="w")
                    nc.vector.tensor_tensor(
                        wt[:], tsl[:, :, None].to_broadcast([128, R, KC]),
                        iota_F[:], Alu.subtract)
                    nc.vector.tensor_scalar(
                        wt[:], wt[:], -1.0, 1.0, Alu.max, Alu.min)
                    sview = wt[:]

                xb = bpool.tile([128, R, KC], bf16, tag="xb")
                nc.vector.tensor_copy(xb[:], xt[:, :, 0:KC])
                arg = apool.tile([128, R, KC], bf16, tag="arg")
                nc.vector.tensor_tensor(arg[:], sview, xb[:], Alu.mult)

                # softplus(a) = ln(exp(a) + 1); "+1" rides the Ln bias.
                et = epool.tile([128, R, KC], bf16, tag="et")
                nc.scalar.activation(et[:], arg[:], Act.Exp)
                spo = apool.tile([128, R, KC], bf16, tag="spo")
                nc.scalar.activation(
                    spo[:], et[:], Act.Ln, bias=1.0,
                    accum_out=acc[:, k:k + 1],
                )

                if k == 2:
                    # overlap the small side-inputs with the tile stream
                    wtile = spool.tile([128, WPER // 128], f32, tag="wts")
                    nc.sync.dma_start(
                        wtile[:], wts.ap().rearrange("(p r) -> p r", p=128))
                    wscr = spool.tile([128, WPER // 128], f32, tag="wts_scr")
                    wacc = fpool.tile([128, 1], f32, tag="wacc")
                    nc.vector.scalar_tensor_tensor(
                        wscr[:], wtile[:], 0.0, wtile[:], Alu.add, Alu.mult,
                        accum_out=wacc[:],
                    )
                    dtile = fpool.tile([1, 192], f32, tag="dt")
                    nc.sync.dma_start(
                        dtile[:], dls.ap().rearrange("(p r) -> p r", p=1))
                    dscr = fpool.tile([1, 192], f32, tag="dscr")
                    dacc = fpool.tile([1, 1], f32, tag="dacc")
                    nc.vector.scalar_tensor_tensor(
                        dscr[:], dtile[:], 0.0, dtile[:], Alu.add, Alu.mult,
                        accum_out=dacc[:],
                    )
                    # N64 per partition: sum of max(t-63, 0)
                    n64scr = fpool.tile([128, RTOT], f32, tag="tall_scr")
                    n64 = fpool.tile([128, 1], f32, tag="n64")
                    nc.vector.tensor_scalar(
                        n64scr[:], tload[:], 63.0, 0.0,
                        Alu.subtract, Alu.max, accum_out=n64[:],
                    )

            # per-partition combine:
            #   comb = accP/B + n64*ln2/B + wacc*alpha/2
            accP = fpool.tile([128, 1], f32, tag="accP")
            nc.vector.reduce_sum(accP[:], acc[:], axis=mybir.AxisListType.X)
            comb = fpool.tile([128, 1], f32, tag="comb")
            nc.vector.tensor_scalar_mul(comb[:], accP[:], 1.0 / B)
            nc.vector.scalar_tensor_tensor(
                comb[:], n64[:], LN2 / B, comb[:], Alu.mult, Alu.add,
            )
            nc.vector.scalar_tensor_tensor(
                comb[:], wacc[:], ALPHA / 2.0, comb[:], Alu.mult, Alu.add,
            )

            # cross-partition sum via matmul with ones, then add delta term
            psum = ppool.tile([1, 1], f32)
            nc.tensor.matmul(psum[:], comb[:], ones[:], start=True, stop=True)
            fin = fpool.tile([1, 1], f32, tag="fin")
            nc.vector.scalar_tensor_tensor(
                fin[:], dacc[:], BETA / 2.0, psum[:], Alu.mult, Alu.add,
            )
            nc.sync.dma_start(out.ap(), fin[:])

    nc.compile()
    return nc


def _get_prog():
    global _PROG
    if _PROG is None:
        _PROG = _build()
    return _PROG


def kernel(logits, targets, weights, deltas):
    from concourse.bass_utils import run_bass_kernel_spmd

    nc = _get_prog()

    lg = np.ascontiguousarray(logits, dtype=np.float32)
    tf = np.ascontiguousarray(targets).astype(np.float32)
    wf = np.ascontiguousarray(weights, dtype=np.float32).reshape(-1)
    d0 = np.zeros(192, dtype=np.float32)
    d0[:189] = np.asarray(deltas, dtype=np.float32)[1:].reshape(-1)
    dz = np.zeros(192, dtype=np.float32)
    in_maps = []
    for c in range(NCORES):
        in_maps.append({
            "logits": lg[c * BC:(c + 1) * BC],
            "targets": tf[c * BC:(c + 1) * BC],
            "wts": wf[c * WPER:(c + 1) * WPER],
            "dls": d0 if c == 0 else dz,
        })

    res = run_bass_kernel_spmd(nc, in_maps, core_ids=list(range(NCORES)))
    total = sum(float(res.results[c]["out"][0, 0]) for c in range(NCORES))
    return np.array(total, dtype=np.float32)
